# revision 38
# baseline (speedup 1.0000x reference)
"""Trainium2 Bass kernel for nn_KnnGraph (topk_masking).

out = affinity * rowtop31mask * coltop31mask, zero diagonal.

Strategy (8 NeuronCores, SPMD), v6:
- Row-shard: core c owns rows [c*R, (c+1)*R). xr is its row slice; xcS is the
  transposed column slice with comb-shuffled columns (xcS[r, s*256+w] =
  A[w*32+s, c*R+r]) so the stride-32 segments used for column candidates are
  contiguous 256-element chunks.
- Thresholds via candidate selection: per 128-line tile, 32 segment top-8s
  (DVE max8), then a 4x max8 + 3x match_replace ladder over the 256
  candidates yields ranks 25..32; rank 31 is the mask threshold.
- Exact tie handling (rank31 == rank32): the rank-31/32 segments are
  re-fetched with [P,1]-offset indirect gathers (one offset per partition is
  all the HW DGE supports) issued inside the scan loops so they overlap;
  the eq*iota + reduce + blend arithmetic is batched across all 8 tiles.
  The kill index is the larger original index of the two copies when tied
  (matching top_k's lowest-index preference). Kill scatters are always-safe
  zero writes (untied kills point at the rank-32 element, which the mask
  already zeroed).
- ONE merged AllGather ships [col thresholds | col tie-kill rows] together;
  row-tile scans and row tie handling run while it is in flight. The mask
  phase re-loads xr so the row-scan x buffers recycle without waiting on
  the collective.
- fp16 output halves store traffic (2^-11 relative, gate is 2e-2; the
  masking compares are fp32-exact).
"""

import os
import sys
from contextlib import ExitStack

import numpy as np

for _p in ("/opt/trn_rl_repo", "/root/.axon_site/_ro/trn_rl_repo"):
    if os.path.isdir(_p) and _p not in sys.path:
        sys.path.append(_p)

import concourse.bass as bass
import concourse.tile as tile
from concourse import bacc, mybir
from concourse.bass import IndirectOffsetOnAxis
from concourse.bass_utils import run_bass_kernel_spmd

P = 128
NEGV = -3.0e38
F32 = mybir.dt.float32
F16 = mybir.dt.float16
U32 = mybir.dt.uint32
ALU = mybir.AluOpType
AX = mybir.AxisListType


def build_nc(N=8192, C=8, iters=1, x_bufs=3, pool_mask_tiles=(),
             no_coll=False, no_kills=False, scan_only=False,
             fuse_mask_tiles=(5, 6, 7)):
    R = N // C            # rows (and cols) per core
    T = R // P            # 128-line tiles per core
    NSEG = 32             # segments per tile line
    SEGW = N // NSEG      # 256
    KF = N // P           # 64: [N] laid out as [P, KF]

    nc = bacc.Bacc(
        "TRN2",
        target_bir_lowering=False,
        debug=False,
        enable_asserts=False,
        num_devices=C,
    )

    xr = nc.dram_tensor("xr", [R, N], F32, kind="ExternalInput")
    xcS = nc.dram_tensor("xcS", [R, N], F32, kind="ExternalInput")
    # constants (f32; integer-valued < 2^24, exact)
    pnf = nc.dram_tensor("pnf", [P, 1], F32, kind="ExternalInput")       # p*N
    pbasef = nc.dram_tensor("pbasef", [P, 1], F32, kind="ExternalInput")  # c*R+p
    kiota = nc.dram_tensor("kiota", [P, KF], F32, kind="ExternalInput")
    dumpdiag = nc.dram_tensor("dumpdiag", [P, KF], F32, kind="ExternalInput")
    iotaseg = nc.dram_tensor("iotaseg", [P, SEGW], F32, kind="ExternalInput")
    qtab2 = nc.dram_tensor("qtab2", [P, 2 * T], F32, kind="ExternalInput")
    qtabr = nc.dram_tensor("qtabr", [P, T], F32, kind="ExternalInput")
    diagk = nc.dram_tensor("diagk", [P, T], U32, kind="ExternalInput")
    out_t = nc.dram_tensor("out", [R, N], F16, kind="ExternalOutput")
    out_flat = out_t.ap().rearrange("a b -> (a b)")[:, None]   # [R*N, 1]
    xr_flat = xr.ap().rearrange("a b -> (a b)")[:, None]
    xcS_flat = xcS.ap().rearrange("a b -> (a b)")[:, None]

    with tile.TileContext(nc) as tc, ExitStack() as ctx:
        xpool = ctx.enter_context(tc.tile_pool(name="x", bufs=x_bufs))
        tpool = ctx.enter_context(tc.tile_pool(name="tmask", bufs=1))
        opool = ctx.enter_context(tc.tile_pool(name="o", bufs=2))
        cpool = ctx.enter_context(tc.tile_pool(name="cand", bufs=1))
        gpool = ctx.enter_context(tc.tile_pool(name="gath", bufs=1))
        spool = ctx.enter_context(tc.tile_pool(name="small", bufs=4))
        stat = ctx.enter_context(tc.tile_pool(name="stat", bufs=1))
        dram = ctx.enter_context(tc.tile_pool(name="dram", bufs=1, space="DRAM"))

        for _it in range(iters):
            m3sc = stat.tile([P, 2 * T], F32, tag="m3sc")
            possc = stat.tile([P, 2 * T], U32, tag="possc")
            m3sr = stat.tile([P, 2 * T], F32, tag="m3sr")
            possr = stat.tile([P, 2 * T], U32, tag="possr")
            rdkill = stat.tile([P, T], U32, tag="rdkill")
            cko = stat.tile([P, KF], U32, tag="cko")
            dgk = stat.tile([P, T], U32, tag="dgk")
            tcbc = tpool.tile([P, N], F32, tag="tcbc")
            kioT = stat.tile([P, KF], F32, tag="kioT")
            ddT = stat.tile([P, KF], F32, tag="ddT")
            pnT = stat.tile([P, 1], F32, tag="pnT")
            pbT = stat.tile([P, 1], F32, tag="pbT")
            segio = stat.tile([P, SEGW], F32, tag="segio")
            qt2 = stat.tile([P, 2 * T], F32, tag="qt2")
            qtr = stat.tile([P, T], F32, tag="qtr")
            zs = stat.tile([P, 1], F16, tag="zs")
            # gather buffers: col pairs [P, 2T*SEGW]; row reuses first half
            gc = gpool.tile([P, 2 * T * SEGW], F32, tag="gc")
            gr = gpool.tile([P, T * SEGW], F32, tag="gr")
            tmpe = gpool.tile([P, 4 * SEGW], F32, tag="tmpe")

            nc.sync.dma_start(kioT[:], kiota.ap())
            nc.sync.dma_start(ddT[:], dumpdiag.ap())
            nc.sync.dma_start(pnT[:], pnf.ap())
            nc.sync.dma_start(pbT[:], pbasef.ap())
            nc.sync.dma_start(segio[:], iotaseg.ap())
            nc.sync.dma_start(qt2[:], qtab2.ap())
            nc.sync.dma_start(qtr[:], qtabr.ap())
            nc.sync.dma_start(dgk[:], diagk.ap())
            nc.gpsimd.memset(zs[:], 0.0)

            def part1(x, m3s, poss, q, copy_eng):
                """Candidate scan on DVE; persist ranks 31/32 (+pos)."""
                cand = cpool.tile([P, NSEG * 8], F32, tag="cand")
                xv = x[:].rearrange("p (s w) -> p s w", s=NSEG)
                for s in range(NSEG):
                    nc.vector.max(cand[:, s * 8:(s + 1) * 8], xv[:, s, :])
                for _ in range(3):
                    m = spool.tile([P, 8], F32, tag="mr")
                    nc.vector.max(m[:], cand[:])
                    nc.vector.match_replace(cand[:], m[:], cand[:], NEGV)
                m3 = spool.tile([P, 8], F32, tag="m3")
                nc.vector.max(m3[:], cand[:])
                pos = spool.tile([P, 8], U32, tag="pos")
                nc.vector.max_index(pos[:], m3[:], cand[:])
                copy_eng.tensor_copy(m3s[:, 2 * q:2 * q + 2], m3[:, 6:8])
                copy_eng.tensor_copy(poss[:, 2 * q:2 * q + 2], pos[:, 6:8])

            def col_gathers(q):
                """Tile q's rank-31/32 segment gathers ([P,1] offsets)."""
                segu = spool.tile([P, 2], U32, tag="cgu")
                nc.vector.tensor_scalar(segu[:], possc[:, 2 * q:2 * q + 2], 3,
                                        None, ALU.logical_shift_right)
                segf = spool.tile([P, 2], F32, tag="cgf")
                nc.gpsimd.tensor_copy(segf[:], segu[:])
                nc.gpsimd.tensor_copy(m3sc_seg[:, 2 * q:2 * q + 2], segf[:])
                offf = spool.tile([P, 2], F32, tag="cgo")
                nc.gpsimd.tensor_scalar(offf[:], segf[:], float(SEGW), None,
                                        ALU.mult)
                nc.gpsimd.tensor_tensor(offf[:], offf[:],
                                        qt2[:, 2 * q:2 * q + 2], ALU.add)
                offu = spool.tile([P, 2], U32, tag="cgq")
                nc.gpsimd.tensor_copy(offu[:], offf[:])
                for j in range(2):
                    k = 2 * q + j
                    nc.gpsimd.indirect_dma_start(
                        out=gc[:, k * SEGW:(k + 1) * SEGW], out_offset=None,
                        in_=xcS_flat,
                        in_offset=IndirectOffsetOnAxis(
                            ap=offu[:, j:j + 1], axis=0),
                    )

            def row_gather(t):
                """Tile t's blended-segment gather ([P,1] offset).

                Row segments are index-monotone, so the tied copy with the
                larger column lives in the larger segment: blend segments
                before the gather; one gather suffices."""
                segu = spool.tile([P, 2], U32, tag="rgu")
                nc.vector.tensor_scalar(segu[:], possr[:, 2 * t:2 * t + 2], 3,
                                        None, ALU.logical_shift_right)
                segf = spool.tile([P, 2], F32, tag="rgf")
                nc.gpsimd.tensor_copy(segf[:], segu[:])
                # tie ? max(s31,s32) : s32
                d = spool.tile([P, 1], F32, tag="rgd")
                nc.gpsimd.tensor_tensor(d[:], m3sr[:, 2 * t:2 * t + 1],
                                        m3sr[:, 2 * t + 1:2 * t + 2],
                                        ALU.subtract)
                nc.gpsimd.tensor_tensor(d[:], d[:], d[:], ALU.mult)
                tie = spool.tile([P, 1], F32, tag="rgt")
                nc.gpsimd.tensor_scalar(tie[:], d[:], 1e-14, None, ALU.is_lt)
                e = spool.tile([P, 1], F32, tag="rge")
                nc.gpsimd.tensor_tensor(e[:], segf[:, 0:1], segf[:, 1:2],
                                        ALU.subtract)
                ip = spool.tile([P, 1], F32, tag="rgi")
                nc.gpsimd.tensor_scalar(ip[:], e[:], 0.0, None, ALU.is_ge)
                nc.gpsimd.tensor_tensor(ip[:], ip[:], tie[:], ALU.mult)
                nc.gpsimd.tensor_tensor(ip[:], ip[:], e[:], ALU.mult)
                sk = spool.tile([P, 1], F32, tag="rgs")
                nc.gpsimd.tensor_tensor(sk[:], segf[:, 1:2], ip[:], ALU.add)
                offf = spool.tile([P, 1], F32, tag="rgo")
                nc.gpsimd.tensor_scalar(offf[:], sk[:], float(SEGW), None,
                                        ALU.mult)
                nc.gpsimd.tensor_tensor(offf[:], offf[:], qtr[:, t:t + 1],
                                        ALU.add)
                nc.gpsimd.tensor_copy(roffs[:, t:t + 1], offf[:])
                offu = spool.tile([P, 1], U32, tag="rgq")
                nc.gpsimd.tensor_copy(offu[:], offf[:])
                nc.gpsimd.indirect_dma_start(
                    out=gr[:, t * SEGW:(t + 1) * SEGW], out_offset=None,
                    in_=xr_flat,
                    in_offset=IndirectOffsetOnAxis(ap=offu[:], axis=0),
                )

            m3sc_seg = stat.tile([P, 2 * T], F32, tag="m3sc_seg")
            roffs = stat.tile([P, T], F32, tag="roffs")
            jlc = spool.tile([P, 2 * T], F32, tag="jlc")

            def col_eq(q):
                """Tile q's eq*iota + reduce (DVE), interleaved with scans."""
                base = (q % 2) * 2 * SEGW
                for j in range(2):
                    k = 2 * q + j
                    nc.vector.scalar_tensor_tensor(
                        tmpe[:, base + j * SEGW:base + (j + 1) * SEGW],
                        gc[:, k * SEGW:(k + 1) * SEGW],
                        m3sc[:, k:k + 1], segio[:], ALU.is_equal, ALU.mult)
                nc.vector.tensor_reduce(
                    jlc[:, 2 * q:2 * q + 2],
                    tmpe[:, base:base + 2 * SEGW].rearrange(
                        "p (k w) -> p k w", k=2),
                    AX.X, ALU.max)

            # ---- phase C: column scans + tie gathers (one tile behind) ----
            for q in range(T):
                x = xpool.tile([P, N], F32, tag="x")
                eng = nc.sync if q % 2 == 0 else nc.scalar
                if q == 0:
                    nc.sync.dma_start(x[:, 0:N // 2], xcS.ap()[0:P, 0:N // 2])
                    nc.scalar.dma_start(x[:, N // 2:N],
                                        xcS.ap()[0:P, N // 2:N])
                else:
                    eng.dma_start(x[:], xcS.ap()[q * P:(q + 1) * P, :])
                part1(x, m3sc, possc, q, nc.gpsimd)
                if not scan_only and q > 0:
                    col_gathers(q - 1)
                    col_eq(q - 1)
            if scan_only:
                o16 = spool.tile([P, 2 * T], F16, tag="o16")
                nc.gpsimd.tensor_copy(o16[:], m3sc[:])
                nc.sync.dma_start(out_t.ap()[0:P, 0:2 * T], o16[:])
                continue
            col_gathers(T - 1)
            col_eq(T - 1)

            # ---- col tie arithmetic tail -> ckrow [P, T] ----
            # original row = w*NSEG + s
            jc = spool.tile([P, 2 * T], F32, tag="jc")
            nc.gpsimd.tensor_scalar(jc[:], jlc[:], float(NSEG), None, ALU.mult)
            nc.gpsimd.tensor_tensor(jc[:], jc[:], m3sc_seg[:], ALU.add)
            # tie ? max(j31, j32) : j32   (batched blend)
            dc = spool.tile([P, T], F32, tag="dc")
            nc.gpsimd.tensor_tensor(dc[:], m3sc[:, 0:2 * T:2],
                                    m3sc[:, 1:2 * T:2], ALU.subtract)
            nc.gpsimd.tensor_tensor(dc[:], dc[:], dc[:], ALU.mult)
            tiec = spool.tile([P, T], F32, tag="tiec")
            nc.gpsimd.tensor_scalar(tiec[:], dc[:], 1e-14, None, ALU.is_lt)
            ec = spool.tile([P, T], F32, tag="ec")
            nc.gpsimd.tensor_tensor(ec[:], jc[:, 0:2 * T:2], jc[:, 1:2 * T:2],
                                    ALU.subtract)
            ipc = spool.tile([P, T], F32, tag="ipc")
            nc.gpsimd.tensor_scalar(ipc[:], ec[:], 0.0, None, ALU.is_ge)
            nc.gpsimd.tensor_tensor(ipc[:], ipc[:], tiec[:], ALU.mult)
            nc.gpsimd.tensor_tensor(ipc[:], ipc[:], ec[:], ALU.mult)
            ckrow = spool.tile([P, T], F32, tag="ckrow")
            nc.gpsimd.tensor_tensor(ckrow[:], jc[:, 1:2 * T:2], ipc[:],
                                    ALU.add)

            # ---- merged collective: [col thresholds | col kill rows] ----
            ship = dram.tile([2 * R], F32, tag="ship")
            gath = dram.tile([C, 2 * R], F32, tag="gath")
            nc.sync.dma_start(ship[0:R].rearrange("(q p) -> p q", p=P),
                              m3sc[:, 0:2 * T:2])
            nc.sync.dma_start(ship[R:2 * R].rearrange("(q p) -> p q", p=P),
                              ckrow[:])
            groups = [list(range(C))]
            if no_coll:
                nc.sync.dma_start(
                    gath[:], ship[:][None].to_broadcast([C, 2 * R]))
            else:
                nc.gpsimd.collective_compute(
                    "AllGather", ALU.bypass, groups,
                    ins=[ship[:].opt()], outs=[gath[:].opt()],
                )

            # ---- row scans + tie gathers (overlap the collective; scan
            # copies on DVE so they don't queue behind it on Pool) ----
            def mask_store(x, t):
                trow = m3sr[:, 2 * t:2 * t + 1]
                o = opool.tile([P, N], F16, tag="o")
                if t in pool_mask_tiles:
                    # 4-op Pool chain per half tile (tt is [P, N/2] to fit
                    # SBUF): T=max(tcbc,trow); d=x-T; m=d>=0; o=m*x
                    H = N // 2
                    for h in range(2):
                        sl = slice(h * H, (h + 1) * H)
                        tt = tpool.tile([P, H], F32, tag="tt")
                        nc.gpsimd.tensor_scalar(tt[:], tcbc[:, sl], trow,
                                                None, ALU.max)
                        nc.gpsimd.tensor_tensor(tt[:], x[:, sl], tt[:],
                                                ALU.subtract)
                        nc.gpsimd.tensor_scalar(tt[:], tt[:], 0.0, None,
                                                ALU.is_ge)
                        nc.gpsimd.tensor_tensor(o[:, sl], tt[:], x[:, sl],
                                                ALU.mult)
                else:
                    nc.vector.scalar_tensor_tensor(o[:], tcbc[:], trow, x[:],
                                                   ALU.max, ALU.is_le)
                    nc.gpsimd.tensor_tensor(o[:], o[:], x[:], ALU.mult)
                nc.sync.dma_start(out_t.ap()[t * P:(t + 1) * P, :], o[:])

            xhold = {}
            for t in range(T):
                x = xpool.tile([P, N], F32, tag="x")
                eng = nc.sync if t % 2 == 0 else nc.scalar
                eng.dma_start(x[:], xr.ap()[t * P:(t + 1) * P, :])
                part1(x, m3sr, possr, t, nc.vector)
                if t > 0:
                    row_gather(t - 1)
                if t in fuse_mask_tiles:
                    xhold[t] = x
            row_gather(T - 1)

            # ---- batched row tie arithmetic -> rdkill [P, T] ----
            jlr = spool.tile([P, T], F32, tag="jlr")
            for h in range(2):
                for kk in range(4):
                    t = 4 * h + kk
                    nc.vector.scalar_tensor_tensor(
                        tmpe[:, kk * SEGW:(kk + 1) * SEGW],
                        gr[:, t * SEGW:(t + 1) * SEGW],
                        m3sr[:, 2 * t + 1:2 * t + 2], segio[:],
                        ALU.is_equal, ALU.mult)
                nc.vector.tensor_reduce(
                    jlr[:, 4 * h:4 * h + 4],
                    tmpe[:].rearrange("p (k w) -> p k w", k=4),
                    AX.X, ALU.max)
            rkf = spool.tile([P, T], F32, tag="rkf")
            nc.gpsimd.tensor_tensor(rkf[:], roffs[:], jlr[:], ALU.add)
            nc.gpsimd.tensor_copy(rdkill[:], rkf[:])

            # ---- post-collective: threshold broadcast + col-kill decode ----
            nc.sync.dma_start(
                tcbc[:], gath[:, 0:R][None].to_broadcast([P, C, R]))
            ck_sb = stat.tile([P, KF], F32, tag="ck_sb")
            nc.sync.dma_start(ck_sb[:], gath[:, R:2 * R])
            t0 = stat.tile([P, 1], F32, tag="t0")
            nc.gpsimd.tensor_scalar(t0[:], pnT[:], 1.0 / N, None, ALU.mult)
            nc.gpsimd.tensor_tensor(t0[:], t0[:], pbT[:], ALU.subtract)  # -c*R
            t1 = stat.tile([P, KF], F32, tag="t1")
            nc.gpsimd.tensor_scalar(t1[:], ck_sb[:], t0[:, 0:1], None, ALU.add)
            v1 = stat.tile([P, KF], F32, tag="v1")
            nc.gpsimd.tensor_scalar(v1[:], t1[:], 0.0, None, ALU.is_ge)
            v2 = stat.tile([P, KF], F32, tag="v2")
            nc.gpsimd.tensor_scalar(v2[:], t1[:], float(R), None, ALU.is_lt)
            nc.gpsimd.tensor_tensor(v1[:], v1[:], v2[:], ALU.mult)
            loc = stat.tile([P, KF], F32, tag="loc")
            nc.gpsimd.tensor_scalar(loc[:], t1[:], float(N), None, ALU.mult)
            nc.gpsimd.tensor_tensor(loc[:], loc[:], kioT[:], ALU.add)
            nc.gpsimd.tensor_tensor(loc[:], loc[:], ddT[:], ALU.subtract)
            nc.gpsimd.tensor_tensor(loc[:], loc[:], v1[:], ALU.mult)
            nc.gpsimd.tensor_tensor(loc[:], loc[:], ddT[:], ALU.add)
            nc.gpsimd.tensor_copy(cko[:], loc[:])

            # ---- fused masks first: these tiles are still resident from
            # the row-scan pass (no re-read); they run the moment the
            # collective lands ----
            for t in sorted(xhold):
                mask_store(xhold[t], t)

            # ---- phase M: mask + store (x re-loaded on the scalar queue;
            # stores keep the sync queue to themselves) ----
            for t in range(T):
                if t in fuse_mask_tiles:
                    continue
                x = xpool.tile([P, N], F32, tag="x")
                nc.scalar.dma_start(x[:], xr.ap()[t * P:(t + 1) * P, :])
                mask_store(x, t)

            # ---- scatter kills (always-safe zero writes) ----
            for t in [] if no_kills else range(T):
                nc.gpsimd.indirect_dma_start(
                    out=out_flat,
                    out_offset=IndirectOffsetOnAxis(ap=dgk[:, t:t + 1], axis=0),
                    in_=zs[:, 0:1], in_offset=None,
                )
            for t in [] if no_kills else range(T):
                nc.gpsimd.indirect_dma_start(
                    out=out_flat,
                    out_offset=IndirectOffsetOnAxis(
                        ap=rdkill[:, t:t + 1], axis=0),
                    in_=zs[:, 0:1], in_offset=None,
                )
            for k in [] if no_kills else range(KF):
                nc.gpsimd.indirect_dma_start(
                    out=out_flat,
                    out_offset=IndirectOffsetOnAxis(ap=cko[:, k:k + 1], axis=0),
                    in_=zs[:, 0:1], in_offset=None,
                )

    nc.compile()
    return nc


_SHUF = None


def _shuffle_perm(N, NSEG=32):
    global _SHUF
    if _SHUF is None or len(_SHUF) != N:
        segw = N // NSEG
        # new position s*segw + w  <- old index w*NSEG + s
        _SHUF = (np.arange(segw)[None, :] * NSEG
                 + np.arange(NSEG)[:, None]).reshape(-1)
    return _SHUF


def make_in_maps(A, N=8192, C=8):
    R = N // C
    KF = N // P
    T = R // P
    SEGW = N // 32
    perm = _shuffle_perm(N)
    g = np.arange(N, dtype=np.float32).reshape(P, KF)         # global col idx
    q = (np.arange(N) % R).astype(np.float32).reshape(P, KF)  # local diag row
    p_ = np.arange(P, dtype=np.float32)
    # qtab2[p, 2q+j] = q*P*N + p*N ; qtabr[p, t] = t*P*N + p*N
    qt2 = (np.repeat(np.arange(T, dtype=np.float32), 2)[None, :] * (P * N)
           + p_[:, None] * N)
    qtr = (np.arange(T, dtype=np.float32)[None, :] * (P * N)
           + p_[:, None] * N)
    in_maps = []
    for c in range(C):
        xcS = np.ascontiguousarray(A[:, c * R:(c + 1) * R].T[:, perm])
        # diag kill offsets: p*N + t*P*N + (c*R + t*P + p)
        dk = (p_[:, None] * N + np.arange(T)[None, :] * (P * N)
              + (c * R + np.arange(T)[None, :] * P + p_[:, None]))
        in_maps.append({
            "xr": np.ascontiguousarray(A[c * R:(c + 1) * R, :]),
            "xcS": xcS,
            "pnf": (p_ * N).reshape(P, 1),
            "pbasef": (c * R + p_).reshape(P, 1),
            "kiota": g,
            "dumpdiag": (q * N + c * R + q).astype(np.float32),
            "iotaseg": np.tile(np.arange(SEGW, dtype=np.float32), (P, 1)),
            "qtab2": qt2.astype(np.float32),
            "qtabr": qtr.astype(np.float32),
            "diagk": dk.astype(np.uint32),
        })
    return in_maps


_NC_CACHE = {}


def kernel(affinity):
    A = np.ascontiguousarray(np.asarray(affinity, dtype=np.float32))
    N = A.shape[0]
    C = 8
    if N not in _NC_CACHE:
        _NC_CACHE[N] = build_nc(N=N, C=C)
    nc = _NC_CACHE[N]
    in_maps = make_in_maps(A, N=N, C=C)
    res = run_bass_kernel_spmd(nc, in_maps, core_ids=list(range(C)))
    outs = res.results
    return np.concatenate(
        [outs[c]["out"].astype(np.float32) for c in range(C)], axis=0)


if __name__ == "__main__":
    A = np.load("/tmp/A.npy")
    got = kernel(A)
    ref = np.load("/tmp/ref_out.npy")
    denom = np.abs(ref).max()
    rel = np.abs(got - ref).max() / denom
    print("differing cells:", int((got != ref).sum()))
    print("Relative error:", rel)
